# revision 20
# baseline (speedup 1.0000x reference)
"""DenseCaptioner LSTM-gate kernel for 8 Trainium2 NeuronCores.

Role-split sharding (no weight replication: each weight matrix is read
from HBM exactly once across the machine):
  cores 0-3  run program VIS: visual + recurrent paths for gate g = core,
             full batch  -> partial logits^T [1024,256]
  cores 4-7  run program INP: input path for gate g = core-4, full batch
             -> partial logits^T [1024,256]
Host: logits[g] = vis_part[g] + inp_part[g] + b[g], then sigmoid/tanh gate
math and the prev_c recurrence.

All matmul operands are bf16 (PSUM accumulation stays fp32): fp32r and
bf16 both stream 1 row/cycle on the TRN2 PE, so bf16's win is purely the
halved HBM traffic (emulated end-to-end rel err 4.6e-3 vs the 2e-2 gate).

Weight-stationary layout: every matmul uses a 128-column slice of the
streamed weight k-tile as the stationary lhsT and the [128,256]
activation k-tile image as the moving rhs, producing outputs directly in
[h-part, batch] layout. Hadamard products stay in that layout and feed
the next level as the moving rhs - no PE transposes, no identity, and
chunk-granular (128-row) pipelining across the hadamard boundaries. The
LDWEIGHTS pipe (8 x 128-row loads per k-tile) double-buffers under the
8 x 256-row matmuls.

Schedule (per core): independent m-projections (C2/U2 resp. W2) run
first into PSUM and are evacuated to SBUF - they keep the PE busy while
the big activation images stream in (activation DMAs are chunked per
4 k-tiles and issued from the otherwise-idle Activation queue so the
sync queue's ~600ns/DMA issue serialization doesn't gate startup), and
the later hadamards multiply PSUM x SBUF directly with no bounce copy.
The U1 stream is emitted between H1's DVE muls and dependent work to
fill that stall. C3+U3 share one open PSUM accumulation group. PSUM
budget: 2 tags x 4 slots x 1 bank (two 256-wide h-chunks per bank tile)
= all 8 banks.

The two programs are dispatched concurrently on disjoint device subsets
through a copy of concourse's PJRT runner that takes an explicit device
list (the stock one hardcodes jax.devices()[:n]).
"""

import numpy as np

import jax
from jax.experimental.shard_map import shard_map
from jax.sharding import Mesh, PartitionSpec

import concourse.mybir as mybir
import concourse.tile as tile
from concourse import bacc, bass2jax

B, X, V, MM, VH, H1, H2, G = 256, 12000, 4096, 1024, 1024, 1024, 1024, 4
XP = 12032  # X padded to a multiple of 128 (94 k-tiles)
N_CORES = 8
NJ = H1 // 128  # output h-chunks per stream

DT_NAME = "bfloat16"  # matmul dtype: "float32r" or "bfloat16"

_cache = {}


def _mm_dt():
    return getattr(mybir.dt, DT_NAME)


def _np_dt():
    return mybir.dt.np(_mm_dt())


def build_program(role):
    """role "vis": visual+recurrent paths; "inp": input path. Full batch."""
    dt = _mm_dt()
    f32 = mybir.dt.float32

    nc = bacc.Bacc("TRN2", target_bir_lowering=False, debug=False)

    if role == "vis":
        act_specs = {"mT": MM, "hT": H2, "v1T": V, "v2T": V}
        w_specs = {"V1": V, "V2": V, "C1": VH, "C2": MM, "C3": H1,
                   "U1": H2, "U2": MM, "U3": H1}
    else:
        act_specs = {"mT": MM, "xT": XP}
        w_specs = {"W1": XP, "W2": MM, "W3": H1}

    acts_d = {
        name: nc.dram_tensor(name, [128, k // 128 * B], dt, kind="ExternalInput")
        for name, k in act_specs.items()
    }
    # weights arrive host-interleaved as k-tile pairs: [K/2, 2*H1], so a
    # [128, 2*H1] tile DMA moves 4KB contiguous per partition
    wt = {
        name: nc.dram_tensor(name, [k // 2, 2 * H1], dt, kind="ExternalInput")
        for name, k in w_specs.items()
    }
    out = nc.dram_tensor("out", [H1, B], f32, kind="ExternalOutput")

    with tile.TileContext(nc) as tc:
        with (
            tc.tile_pool(name="acts", bufs=1) as acts,
            tc.tile_pool(name="wstream", bufs=8) as wstream,
            tc.tile_pool(name="inter", bufs=1) as inter,
            tc.tile_pool(name="ps", bufs=4, space="PSUM") as ps,
        ):
            # PE warmup: ~32 dependency-free matmuls on zeroed SBUF start
            # the DVFS ramp during the DMA head instead of after it
            warm = acts.tile([128, 2 * B], dt, tag="warm")
            nc.vector.memset(warm[:], 0.0)
            wps = ps.tile([128, 2 * B], f32, tag="A", name="warm_ps")
            for i in range(32):
                nc.tensor.matmul(
                    wps[:, :B], warm[:, :128], warm[:, :B],
                    start=(i == 0), stop=(i == 31),
                )

            act_sb = {}

            def load_act(name, chunk_kt=4):
                """Chunked resident activation load, [128, ktile, batch]
                image; issued on the Activation queue."""
                dram = acts_d[name]
                ktiles = act_specs[name] // 128
                t = acts.tile([128, ktiles * B], dt, tag=name, name=name)
                for c0 in range(0, ktiles, chunk_kt):
                    c1 = min(c0 + chunk_kt, ktiles)
                    nc.scalar.dma_start(
                        t[:, c0 * B:c1 * B], dram.ap()[:, c0 * B:c1 * B]
                    )
                act_sb[name] = t.rearrange("p (t x) -> p t x", x=B)

            def act_view(name):
                return lambda k: act_sb[name][:, k, :]

            def q_view(q):
                return lambda k: q[:, k * B:(k + 1) * B]

            # psum: [128, 512] f32 bank tiles, two 256-wide h-chunks each
            def pslice(psums, j):
                return psums[j // 2][:, (j % 2) * B:(j % 2 + 1) * B]

            def stream_mm(rhs, wname, ptag, psums=None, start_group=True,
                          stop_group=True, j_outer=False):
                """pslice(psums, j) [128, 256] (+)= W_ktile[:, j*128:...]^T
                @ rhs(k), streaming W k-tiles: weight columns stationary,
                activation image moving. Weights arrive as k-tile PAIRS
                (host-interleaved so each DMA descriptor is a contiguous
                4KB). j_outer completes output chunks progressively (for
                the final stream, so evac/store overlap the matmul tail)
                with all pair-tiles staged first."""
                ktiles = w_specs[wname] // 128
                w_dram = wt[wname].ap().rearrange("(t p) n -> t p n", p=128)
                if psums is None:
                    psums = [
                        ps.tile([128, 2 * B], f32, tag=ptag,
                                name=f"ps_{wname}{i}")
                        for i in range(NJ // 2)
                    ]

                def mm(k, j, w):
                    nc.tensor.matmul(
                        pslice(psums, j),
                        w[:, (k % 2) * H1 + j * 128:(k % 2) * H1 + (j + 1) * 128],
                        rhs(k),
                        # start zeroes the whole 2KB PSUM bank, so only the
                        # first write of each bank pair may set it
                        start=start_group and (k == 0) and (j % 2 == 0),
                        stop=stop_group and (k == ktiles - 1),
                    )

                if j_outer:
                    ws = []
                    for k2 in range(ktiles // 2):
                        w = wstream.tile([128, 2 * H1], dt, tag="w",
                                         name=f"w_{wname}{k2}")
                        nc.sync.dma_start(w[:], w_dram[k2])
                        ws.append(w)
                    for j in range(NJ):
                        for k in range(ktiles):
                            mm(k, j, ws[k // 2])
                else:
                    for k2 in range(ktiles // 2):
                        w = wstream.tile([128, 2 * H1], dt, tag="w",
                                         name=f"w_{wname}{k2}")
                        nc.sync.dma_start(w[:], w_dram[k2])
                        for k in (2 * k2, 2 * k2 + 1):
                            for j in range(NJ):
                                mm(k, j, w)
                return psums

            def evac_sbuf(psums, name):
                """Copy psum accumulators to a resident SBUF f32 image,
                alternating Vector/Activation engines to halve latency."""
                s = inter.tile([128, NJ * B], f32, tag=name, name=name)
                for j in range(NJ):
                    dst = s[:, j * B:(j + 1) * B]
                    if j % 2 == 0:
                        nc.vector.tensor_copy(dst, pslice(psums, j))
                    else:
                        nc.scalar.copy(dst, pslice(psums, j))
                return s

            def had_mul(pa, partner_sb=None, bounce_from=None, qname="q"):
                """q [128, NJ*256] bf16 = pa * partner, chunk-granular.
                partner: resident SBUF f32 image, or psum bounced via SBUF."""
                q = inter.tile([128, NJ * B], dt, tag="q", name=qname, bufs=2)
                bnc = None
                if partner_sb is None:
                    bnc = inter.tile([128, NJ * B], f32, tag="bounce",
                                     name=f"bounce_{qname}", bufs=2)
                for j in range(NJ):
                    sl = slice(j * B, (j + 1) * B)
                    if partner_sb is None:
                        # bounce on the Activation engine so the Vector muls
                        # pipeline right behind it
                        nc.scalar.copy(bnc[:, sl], pslice(bounce_from, j))
                        src = bnc[:, sl]
                    else:
                        src = partner_sb[:, sl]
                    nc.vector.tensor_mul(q[:, sl], pslice(pa, j), src)
                return q

            def finish(l3):
                acc = inter.tile([128, NJ * B], f32, tag="acc", name="acc")
                out_v = out.ap().rearrange("(j p) b -> j p b", p=128)
                for j in range(NJ):
                    sl = slice(j * B, (j + 1) * B)
                    if j % 2 == 0:
                        nc.vector.tensor_copy(acc[:, sl], pslice(l3, j))
                    else:
                        nc.scalar.copy(acc[:, sl], pslice(l3, j))
                    # sync queue is idle by now; keeps stores off the
                    # Activation queue which is doing the copies
                    nc.sync.dma_start(out_v[j], acc[:, sl])

            if role == "vis":
                load_act("mT")
                pc2 = stream_mm(act_view("mT"), "C2", "A")
                c2sb = evac_sbuf(pc2, "c2sb")
                load_act("hT")
                pu2 = stream_mm(act_view("mT"), "U2", "B")
                u2sb = evac_sbuf(pu2, "u2sb")
                load_act("v1T", chunk_kt=8)
                load_act("v2T", chunk_kt=8)
                pa = stream_mm(act_view("v1T"), "V1", "A")
                pb = stream_mm(act_view("v2T"), "V2", "B")
                q1 = had_mul(pa, bounce_from=pb, qname="q1")  # frees A and B
                # U1 in B: only waits H1's bounce copies, not its muls
                pu = stream_mm(act_view("hT"), "U1", "B")     # fills H1 stall
                pa2 = stream_mm(q_view(q1), "C1", "A")
                q2 = had_mul(pa2, partner_sb=c2sb, qname="q2")  # frees A
                l3 = stream_mm(q_view(q2), "C3", "A", start_group=True,
                               stop_group=False)
                q3 = had_mul(pu, partner_sb=u2sb, qname="q3")  # frees B; || C3
                stream_mm(q_view(q3), "U3", None, psums=l3,
                          start_group=False, stop_group=True, j_outer=True)
                finish(l3)
            else:
                load_act("mT")
                pw2 = stream_mm(act_view("mT"), "W2", "A")
                w2sb = evac_sbuf(pw2, "w2sb")
                load_act("xT", chunk_kt=8)
                pa = stream_mm(act_view("xT"), "W1", "B")
                q = had_mul(pa, partner_sb=w2sb, qname="q1")   # frees B
                # k-inner: W3 row k consumes q chunk k as the muls emit them
                l3 = stream_mm(q_view(q), "W3", "A")
                finish(l3)

    nc.compile()
    return nc


def _make_runner(nc, devices):
    """Adapted from concourse.bass2jax.run_bass_via_pjrt: same lowering,
    but runs on an explicit device subset and returns unmaterialized jax
    arrays so two programs can be dispatched concurrently."""
    bass2jax.install_neuronx_cc_hook()

    assert nc.dbg_addr is None
    partition_name = (
        nc.partition_id_tensor.name if nc.partition_id_tensor else None
    )

    in_names, out_names, out_avals, zero_outs = [], [], [], []
    for alloc in nc.m.functions[0].allocations:
        if not isinstance(alloc, mybir.MemoryLocationSet):
            continue
        name = alloc.memorylocations[0].name
        if alloc.kind == "ExternalInput":
            if name != partition_name:
                in_names.append(name)
        elif alloc.kind == "ExternalOutput":
            shape = tuple(alloc.tensor_shape)
            dtype = mybir.dt.np(alloc.dtype)
            out_names.append(name)
            out_avals.append(jax.core.ShapedArray(shape, dtype))
            zero_outs.append(np.zeros(shape, dtype))
    n_params = len(in_names)
    n_outs = len(out_avals)
    in_names.extend(out_names)
    if partition_name is not None:
        in_names.append(partition_name)
    donate = tuple(range(n_params, n_params + n_outs))

    def _body(*args):
        operands = list(args)
        if partition_name is not None:
            operands.append(bass2jax.partition_id_tensor())
        outs = bass2jax._bass_exec_p.bind(
            *operands,
            out_avals=tuple(out_avals),
            in_names=tuple(in_names),
            out_names=tuple(out_names),
            lowering_input_output_aliases=(),
            sim_require_finite=True,
            sim_require_nnan=True,
            nc=nc,
        )
        return tuple(outs)

    n_cores = len(devices)
    mesh = Mesh(np.asarray(devices), ("core",))
    in_specs = (PartitionSpec("core"),) * (n_params + n_outs)
    out_specs = (PartitionSpec("core"),) * n_outs
    sharded = jax.jit(
        shard_map(
            _body, mesh=mesh, in_specs=in_specs, out_specs=out_specs,
            check_rep=False,
        ),
        donate_argnums=donate,
        keep_unused=True,
    )

    def run(in_maps):
        assert len(in_maps) == n_cores
        concat_in = [
            np.concatenate(
                [np.asarray(in_maps[c][name]) for c in range(n_cores)], axis=0
            )
            for name in in_names[:n_params]
        ]
        concat_zeros = [
            np.zeros((n_cores * z.shape[0], *z.shape[1:]), z.dtype)
            for z in zero_outs
        ]
        out_arrs = sharded(*concat_in, *concat_zeros)
        return out_names, out_avals, out_arrs

    return run


def _wpair(w):
    """[K, H] -> [K/2, 2H]: interleave k-tile pairs so each partition's DMA
    line is 4KB contiguous (rows p and p+128 of a tile pair adjacent)."""
    wk = np.asarray(w, np.float32)
    K_, H = wk.shape
    kt = K_ // 128
    r = wk.reshape(kt // 2, 2, 128, H).transpose(0, 2, 1, 3)
    return np.ascontiguousarray(r.reshape(K_ // 2, 2 * H), dtype=_np_dt())


def _tile_actT(a, kdim):
    """[256 batch, K<=kdim] -> SBUF image [128, (kdim/128) * 256]:
    (p, t*256+b) = a[b, t*128+p], contiguous per partition."""
    ktiles = kdim // 128
    a = np.asarray(a, np.float32)
    if a.shape[1] < kdim:
        a = np.pad(a, ((0, 0), (0, kdim - a.shape[1])))
    # [256b, ktiles, 128p] -> [128p, ktiles, 256b]
    r = a.reshape(B, ktiles, 128).transpose(2, 1, 0)
    return np.ascontiguousarray(r.reshape(128, ktiles * B), dtype=_np_dt())


def kernel(prev_h, prev_c, x, m, v1, v2, V1, V2, C1, C2, C3, W1, W2, W3, U1, U2, U3, b):
    npdt = _np_dt()
    if "runners" not in _cache:
        devs = jax.devices()
        nc_vis = build_program("vis")
        nc_inp = build_program("inp")
        _cache["runners"] = (
            _make_runner(nc_vis, devs[0:4]),
            _make_runner(nc_inp, devs[4:8]),
        )
        _cache["ncs"] = (nc_vis, nc_inp)
    run_vis, run_inp = _cache["runners"]

    v1T_img = _tile_actT(v1, V)
    v2T_img = _tile_actT(v2, V)
    mT_img = _tile_actT(m, MM)
    hT_img = _tile_actT(prev_h, H2)
    xT_img = _tile_actT(x, XP)

    vis_maps, inp_maps = [], []
    for g in range(G):
        vis_maps.append({
            "v1T": v1T_img, "v2T": v2T_img, "mT": mT_img, "hT": hT_img,
            "V1": _wpair(V1[g]), "V2": _wpair(V2[g]), "C1": _wpair(C1[g]),
            "C2": _wpair(C2[g]), "C3": _wpair(C3[g]), "U1": _wpair(U1[g]),
            "U2": _wpair(U2[g]), "U3": _wpair(U3[g]),
        })
        w1_pad = np.zeros((XP, H1), np.float32)
        w1_pad[:X] = np.asarray(W1[g], np.float32)
        inp_maps.append({
            "xT": xT_img, "mT": mT_img,
            "W1": _wpair(w1_pad),
            "W2": _wpair(W2[g]), "W3": _wpair(W3[g]),
        })

    _cache["last_in_maps"] = (vis_maps, inp_maps)

    # dispatch both programs; they run concurrently on disjoint cores
    vnames, vavals, vouts = run_vis(vis_maps)
    inames, iavals, iouts = run_inp(inp_maps)

    # outputs are logits^T [G, H2, B]
    vis_out = np.asarray(vouts[0]).reshape(G, H1, B)
    inp_out = np.asarray(iouts[0]).reshape(G, H1, B)

    logits = (vis_out + inp_out).transpose(0, 2, 1) + \
        np.asarray(b, np.float32)[:, None, :]

    def sigmoid(z):
        return 1.0 / (1.0 + np.exp(-z))

    i = sigmoid(logits[0])
    f = sigmoid(logits[1])
    o = sigmoid(logits[2])
    cg = np.tanh(logits[3])
    prev_c = np.asarray(prev_c, np.float32)
    new_c = f * prev_c + i * cg
    new_h = o * np.tanh(prev_c)
    return new_h.astype(np.float32), new_c.astype(np.float32)


# revision 22
# speedup vs baseline: 1.0073x; 1.0073x over previous
"""DenseCaptioner LSTM-gate kernel for 8 Trainium2 NeuronCores.

Role-split sharding (no weight replication: each weight matrix is read
from HBM exactly once across the machine):
  cores 0-3  run program VIS: visual + recurrent paths for gate g = core,
             full batch  -> partial logits^T [1024,256]
  cores 4-7  run program INP: input path for gate g = core-4, full batch
             -> partial logits^T [1024,256]
Host: logits[g] = vis_part[g] + inp_part[g] + b[g], then sigmoid/tanh gate
math and the prev_c recurrence.

All matmul operands are bf16 (PSUM accumulation stays fp32): fp32r and
bf16 both stream 1 row/cycle on the TRN2 PE, so bf16's win is purely the
halved HBM traffic (emulated end-to-end rel err 4.6e-3 vs the 2e-2 gate).

Weight-stationary layout: every matmul uses a 128-column slice of the
streamed weight k-tile as the stationary lhsT and the [128,256]
activation k-tile image as the moving rhs, producing outputs directly in
[h-part, batch] layout. Hadamard products stay in that layout and feed
the next level as the moving rhs - no PE transposes, no identity, and
chunk-granular (128-row) pipelining across the hadamard boundaries. The
LDWEIGHTS pipe (8 x 128-row loads per k-tile) double-buffers under the
8 x 256-row matmuls.

Schedule (per core): independent m-projections (C2/U2 resp. W2) run
first into PSUM and are evacuated to SBUF - they keep the PE busy while
the big activation images stream in (activation DMAs are chunked per
4 k-tiles and issued from the otherwise-idle Activation queue so the
sync queue's ~600ns/DMA issue serialization doesn't gate startup), and
the later hadamards multiply PSUM x SBUF directly with no bounce copy.
The U1 stream is emitted between H1's DVE muls and dependent work to
fill that stall. C3+U3 share one open PSUM accumulation group. PSUM
budget: 2 tags x 4 slots x 1 bank (two 256-wide h-chunks per bank tile)
= all 8 banks.

The two programs are dispatched concurrently on disjoint device subsets
through a copy of concourse's PJRT runner that takes an explicit device
list (the stock one hardcodes jax.devices()[:n]).
"""

import numpy as np

import jax
from jax.experimental.shard_map import shard_map
from jax.sharding import Mesh, PartitionSpec

import concourse.mybir as mybir
import concourse.tile as tile
from concourse import bacc, bass2jax

B, X, V, MM, VH, H1, H2, G = 256, 12000, 4096, 1024, 1024, 1024, 1024, 4
XP = 12032  # X padded to a multiple of 128 (94 k-tiles)
N_CORES = 8
NJ = H1 // 128  # output h-chunks per stream

DT_NAME = "bfloat16"  # matmul dtype: "float32r" or "bfloat16"

_cache = {}


def _mm_dt():
    return getattr(mybir.dt, DT_NAME)


def _np_dt():
    return mybir.dt.np(_mm_dt())


def build_program(role):
    """role "vis": visual+recurrent paths; "inp": input path. Full batch."""
    dt = _mm_dt()
    f32 = mybir.dt.float32

    nc = bacc.Bacc("TRN2", target_bir_lowering=False, debug=False)

    if role == "vis":
        act_specs = {"mT": MM, "hT": H2, "v1T": V, "v2T": V}
        w_specs = {"V1": V, "V2": V, "C1": VH, "C2": MM, "C3": H1,
                   "U1": H2, "U2": MM, "U3": H1}
    else:
        act_specs = {"mT": MM, "xT": XP}
        w_specs = {"W1": XP, "W2": MM, "W3": H1}

    acts_d = {
        name: nc.dram_tensor(name, [128, k // 128 * B], dt, kind="ExternalInput")
        for name, k in act_specs.items()
    }
    # weights arrive host-interleaved as k-tile pairs: [K/2, 2*H1], so a
    # [128, 2*H1] tile DMA moves 4KB contiguous per partition
    wt = {
        name: nc.dram_tensor(name, [k // 2, 2 * H1], dt, kind="ExternalInput")
        for name, k in w_specs.items()
    }
    out = nc.dram_tensor("out", [H1, B], f32, kind="ExternalOutput")

    with tile.TileContext(nc) as tc:
        with (
            tc.tile_pool(name="acts", bufs=1) as acts,
            tc.tile_pool(name="wstream", bufs=10) as wstream,
            tc.tile_pool(name="inter", bufs=1) as inter,
            tc.tile_pool(name="ps", bufs=4, space="PSUM") as ps,
        ):
            # PE warmup: ~32 dependency-free matmuls on zeroed SBUF start
            # the DVFS ramp during the DMA head instead of after it
            warm = acts.tile([128, 2 * B], dt, tag="warm")
            nc.vector.memset(warm[:], 0.0)
            wps = ps.tile([128, 2 * B], f32, tag="A", name="warm_ps")
            for i in range(16):
                nc.tensor.matmul(
                    wps[:, :B], warm[:, :128], warm[:, :B],
                    start=(i == 0), stop=(i == 15),
                )

            act_sb = {}

            def load_act(name, chunk_kt=4):
                """Chunked resident activation load, [128, ktile, batch]
                image; issued on the Activation queue."""
                dram = acts_d[name]
                ktiles = act_specs[name] // 128
                t = acts.tile([128, ktiles * B], dt, tag=name, name=name)
                for c0 in range(0, ktiles, chunk_kt):
                    c1 = min(c0 + chunk_kt, ktiles)
                    nc.scalar.dma_start(
                        t[:, c0 * B:c1 * B], dram.ap()[:, c0 * B:c1 * B]
                    )
                act_sb[name] = t.rearrange("p (t x) -> p t x", x=B)

            def act_view(name):
                return lambda k: act_sb[name][:, k, :]

            def q_view(q):
                return lambda k: q[:, k * B:(k + 1) * B]

            # psum: [128, 512] f32 bank tiles, two 256-wide h-chunks each
            def pslice(psums, j):
                return psums[j // 2][:, (j % 2) * B:(j % 2 + 1) * B]

            def stream_mm(rhs, wname, ptag, psums=None, start_group=True,
                          stop_group=True, j_outer=False):
                """pslice(psums, j) [128, 256] (+)= W_ktile[:, j*128:...]^T
                @ rhs(k), streaming W k-tiles: weight columns stationary,
                activation image moving. Weights arrive as k-tile PAIRS
                (host-interleaved so each DMA descriptor is a contiguous
                4KB). j_outer completes output chunks progressively (for
                the final stream, so evac/store overlap the matmul tail)
                with all pair-tiles staged first."""
                ktiles = w_specs[wname] // 128
                w_dram = wt[wname].ap().rearrange("(t p) n -> t p n", p=128)
                if psums is None:
                    psums = [
                        ps.tile([128, 2 * B], f32, tag=ptag,
                                name=f"ps_{wname}{i}")
                        for i in range(NJ // 2)
                    ]

                def mm(k, j, w):
                    nc.tensor.matmul(
                        pslice(psums, j),
                        w[:, (k % 2) * H1 + j * 128:(k % 2) * H1 + (j + 1) * 128],
                        rhs(k),
                        # start zeroes the whole 2KB PSUM bank, so only the
                        # first write of each bank pair may set it
                        start=start_group and (k == 0) and (j % 2 == 0),
                        stop=stop_group and (k == ktiles - 1),
                    )

                if j_outer:
                    ws = []
                    for k2 in range(ktiles // 2):
                        w = wstream.tile([128, 2 * H1], dt, tag="w",
                                         name=f"w_{wname}{k2}")
                        nc.sync.dma_start(w[:], w_dram[k2])
                        ws.append(w)
                    for j in range(NJ):
                        for k in range(ktiles):
                            mm(k, j, ws[k // 2])
                else:
                    for k2 in range(ktiles // 2):
                        w = wstream.tile([128, 2 * H1], dt, tag="w",
                                         name=f"w_{wname}{k2}")
                        nc.sync.dma_start(w[:], w_dram[k2])
                        for k in (2 * k2, 2 * k2 + 1):
                            for j in range(NJ):
                                mm(k, j, w)
                return psums

            def evac_sbuf(psums, name):
                """Copy psum accumulators to a resident SBUF f32 image,
                alternating Vector/Activation engines to halve latency."""
                s = inter.tile([128, NJ * B], f32, tag=name, name=name)
                for j in range(NJ):
                    dst = s[:, j * B:(j + 1) * B]
                    if j % 2 == 0:
                        nc.vector.tensor_copy(dst, pslice(psums, j))
                    else:
                        nc.scalar.copy(dst, pslice(psums, j))
                return s

            def had_mul(pa, partner_sb=None, bounce_from=None, qname="q"):
                """q [128, NJ*256] bf16 = pa * partner, chunk-granular.
                partner: resident SBUF f32 image, or psum bounced via SBUF."""
                q = inter.tile([128, NJ * B], dt, tag="q", name=qname, bufs=2)
                bnc = None
                if partner_sb is None:
                    bnc = inter.tile([128, NJ * B], f32, tag="bounce",
                                     name=f"bounce_{qname}", bufs=2)
                for j in range(NJ):
                    sl = slice(j * B, (j + 1) * B)
                    if partner_sb is None:
                        # bounce on the Activation engine so the Vector muls
                        # pipeline right behind it
                        nc.scalar.copy(bnc[:, sl], pslice(bounce_from, j))
                        src = bnc[:, sl]
                    else:
                        src = partner_sb[:, sl]
                    nc.vector.tensor_mul(q[:, sl], pslice(pa, j), src)
                return q

            def finish(l3):
                acc = inter.tile([128, NJ * B], f32, tag="acc", name="acc")
                out_v = out.ap().rearrange("(j p) b -> j p b", p=128)
                for j in range(NJ):
                    sl = slice(j * B, (j + 1) * B)
                    if j % 2 == 0:
                        nc.vector.tensor_copy(acc[:, sl], pslice(l3, j))
                    else:
                        nc.scalar.copy(acc[:, sl], pslice(l3, j))
                    # sync queue is idle by now; keeps stores off the
                    # Activation queue which is doing the copies
                    nc.sync.dma_start(out_v[j], acc[:, sl])

            if role == "vis":
                load_act("mT")
                pc2 = stream_mm(act_view("mT"), "C2", "A")
                c2sb = evac_sbuf(pc2, "c2sb")
                load_act("hT")
                pu2 = stream_mm(act_view("mT"), "U2", "B")
                u2sb = evac_sbuf(pu2, "u2sb")
                load_act("v1T", chunk_kt=8)
                load_act("v2T", chunk_kt=8)
                pa = stream_mm(act_view("v1T"), "V1", "A")
                pb = stream_mm(act_view("v2T"), "V2", "B")
                q1 = had_mul(pa, bounce_from=pb, qname="q1")  # frees A and B
                # U1 in B: only waits H1's bounce copies, not its muls
                pu = stream_mm(act_view("hT"), "U1", "B")     # fills H1 stall
                pa2 = stream_mm(q_view(q1), "C1", "A")
                q2 = had_mul(pa2, partner_sb=c2sb, qname="q2")  # frees A
                l3 = stream_mm(q_view(q2), "C3", "A", start_group=True,
                               stop_group=False)
                q3 = had_mul(pu, partner_sb=u2sb, qname="q3")  # frees B; || C3
                stream_mm(q_view(q3), "U3", None, psums=l3,
                          start_group=False, stop_group=True, j_outer=True)
                finish(l3)
            else:
                load_act("mT")
                pw2 = stream_mm(act_view("mT"), "W2", "A")
                w2sb = evac_sbuf(pw2, "w2sb")
                load_act("xT", chunk_kt=8)
                pa = stream_mm(act_view("xT"), "W1", "B")
                q = had_mul(pa, partner_sb=w2sb, qname="q1")   # frees B
                # k-inner: W3 row k consumes q chunk k as the muls emit them
                l3 = stream_mm(q_view(q), "W3", "A")
                finish(l3)

    nc.compile()
    return nc


def _make_runner(nc, devices):
    """Adapted from concourse.bass2jax.run_bass_via_pjrt: same lowering,
    but runs on an explicit device subset and returns unmaterialized jax
    arrays so two programs can be dispatched concurrently."""
    bass2jax.install_neuronx_cc_hook()

    assert nc.dbg_addr is None
    partition_name = (
        nc.partition_id_tensor.name if nc.partition_id_tensor else None
    )

    in_names, out_names, out_avals, zero_outs = [], [], [], []
    for alloc in nc.m.functions[0].allocations:
        if not isinstance(alloc, mybir.MemoryLocationSet):
            continue
        name = alloc.memorylocations[0].name
        if alloc.kind == "ExternalInput":
            if name != partition_name:
                in_names.append(name)
        elif alloc.kind == "ExternalOutput":
            shape = tuple(alloc.tensor_shape)
            dtype = mybir.dt.np(alloc.dtype)
            out_names.append(name)
            out_avals.append(jax.core.ShapedArray(shape, dtype))
            zero_outs.append(np.zeros(shape, dtype))
    n_params = len(in_names)
    n_outs = len(out_avals)
    in_names.extend(out_names)
    if partition_name is not None:
        in_names.append(partition_name)
    donate = tuple(range(n_params, n_params + n_outs))

    def _body(*args):
        operands = list(args)
        if partition_name is not None:
            operands.append(bass2jax.partition_id_tensor())
        outs = bass2jax._bass_exec_p.bind(
            *operands,
            out_avals=tuple(out_avals),
            in_names=tuple(in_names),
            out_names=tuple(out_names),
            lowering_input_output_aliases=(),
            sim_require_finite=True,
            sim_require_nnan=True,
            nc=nc,
        )
        return tuple(outs)

    n_cores = len(devices)
    mesh = Mesh(np.asarray(devices), ("core",))
    in_specs = (PartitionSpec("core"),) * (n_params + n_outs)
    out_specs = (PartitionSpec("core"),) * n_outs
    sharded = jax.jit(
        shard_map(
            _body, mesh=mesh, in_specs=in_specs, out_specs=out_specs,
            check_rep=False,
        ),
        donate_argnums=donate,
        keep_unused=True,
    )

    def run(in_maps):
        assert len(in_maps) == n_cores
        concat_in = [
            np.concatenate(
                [np.asarray(in_maps[c][name]) for c in range(n_cores)], axis=0
            )
            for name in in_names[:n_params]
        ]
        concat_zeros = [
            np.zeros((n_cores * z.shape[0], *z.shape[1:]), z.dtype)
            for z in zero_outs
        ]
        out_arrs = sharded(*concat_in, *concat_zeros)
        return out_names, out_avals, out_arrs

    return run


def _wpair(w):
    """[K, H] -> [K/2, 2H]: interleave k-tile pairs so each partition's DMA
    line is 4KB contiguous (rows p and p+128 of a tile pair adjacent)."""
    wk = np.asarray(w, np.float32)
    K_, H = wk.shape
    kt = K_ // 128
    r = wk.reshape(kt // 2, 2, 128, H).transpose(0, 2, 1, 3)
    return np.ascontiguousarray(r.reshape(K_ // 2, 2 * H), dtype=_np_dt())


def _tile_actT(a, kdim):
    """[256 batch, K<=kdim] -> SBUF image [128, (kdim/128) * 256]:
    (p, t*256+b) = a[b, t*128+p], contiguous per partition."""
    ktiles = kdim // 128
    a = np.asarray(a, np.float32)
    if a.shape[1] < kdim:
        a = np.pad(a, ((0, 0), (0, kdim - a.shape[1])))
    # [256b, ktiles, 128p] -> [128p, ktiles, 256b]
    r = a.reshape(B, ktiles, 128).transpose(2, 1, 0)
    return np.ascontiguousarray(r.reshape(128, ktiles * B), dtype=_np_dt())


def kernel(prev_h, prev_c, x, m, v1, v2, V1, V2, C1, C2, C3, W1, W2, W3, U1, U2, U3, b):
    npdt = _np_dt()
    if "runners" not in _cache:
        devs = jax.devices()
        nc_vis = build_program("vis")
        nc_inp = build_program("inp")
        _cache["runners"] = (
            _make_runner(nc_vis, devs[0:4]),
            _make_runner(nc_inp, devs[4:8]),
        )
        _cache["ncs"] = (nc_vis, nc_inp)
    run_vis, run_inp = _cache["runners"]

    v1T_img = _tile_actT(v1, V)
    v2T_img = _tile_actT(v2, V)
    mT_img = _tile_actT(m, MM)
    hT_img = _tile_actT(prev_h, H2)
    xT_img = _tile_actT(x, XP)

    vis_maps, inp_maps = [], []
    for g in range(G):
        vis_maps.append({
            "v1T": v1T_img, "v2T": v2T_img, "mT": mT_img, "hT": hT_img,
            "V1": _wpair(V1[g]), "V2": _wpair(V2[g]), "C1": _wpair(C1[g]),
            "C2": _wpair(C2[g]), "C3": _wpair(C3[g]), "U1": _wpair(U1[g]),
            "U2": _wpair(U2[g]), "U3": _wpair(U3[g]),
        })
        w1_pad = np.zeros((XP, H1), np.float32)
        w1_pad[:X] = np.asarray(W1[g], np.float32)
        inp_maps.append({
            "xT": xT_img, "mT": mT_img,
            "W1": _wpair(w1_pad),
            "W2": _wpair(W2[g]), "W3": _wpair(W3[g]),
        })

    _cache["last_in_maps"] = (vis_maps, inp_maps)

    # dispatch both programs; they run concurrently on disjoint cores
    vnames, vavals, vouts = run_vis(vis_maps)
    inames, iavals, iouts = run_inp(inp_maps)

    # outputs are logits^T [G, H2, B]
    vis_out = np.asarray(vouts[0]).reshape(G, H1, B)
    inp_out = np.asarray(iouts[0]).reshape(G, H1, B)

    logits = (vis_out + inp_out).transpose(0, 2, 1) + \
        np.asarray(b, np.float32)[:, None, :]

    def sigmoid(z):
        return 1.0 / (1.0 + np.exp(-z))

    i = sigmoid(logits[0])
    f = sigmoid(logits[1])
    o = sigmoid(logits[2])
    cg = np.tanh(logits[3])
    prev_c = np.asarray(prev_c, np.float32)
    new_c = f * prev_c + i * cg
    new_h = o * np.tanh(prev_c)
    return new_h.astype(np.float32), new_c.astype(np.float32)


# revision 26
# speedup vs baseline: 1.0212x; 1.0139x over previous
"""DenseCaptioner LSTM-gate kernel for 8 Trainium2 NeuronCores.

Role-split sharding (no weight replication: each weight matrix is read
from HBM exactly once across the machine):
  cores 0-3  run program VIS: visual + recurrent paths for gate g = core,
             full batch  -> partial logits^T [1024,256]
  cores 4-7  run program INP: input path for gate g = core-4, full batch
             -> partial logits^T [1024,256]
Host: logits[g] = vis_part[g] + inp_part[g] + b[g], then sigmoid/tanh gate
math and the prev_c recurrence.

All matmul operands are bf16 (PSUM accumulation stays fp32): fp32r and
bf16 both stream 1 row/cycle on the TRN2 PE, so bf16's win is purely the
halved HBM traffic (emulated end-to-end rel err 4.6e-3 vs the 2e-2 gate).

Weight-stationary layout: every matmul uses a 128-column slice of the
streamed weight k-tile as the stationary lhsT and the [128,256]
activation k-tile image as the moving rhs, producing outputs directly in
[h-part, batch] layout. Hadamard products stay in that layout and feed
the next level as the moving rhs - no PE transposes, no identity, and
chunk-granular (128-row) pipelining across the hadamard boundaries. The
LDWEIGHTS pipe (8 x 128-row loads per k-tile) double-buffers under the
8 x 256-row matmuls.

Schedule (per core): independent m-projections (C2/U2 resp. W2) run
first into PSUM and are evacuated to SBUF - they keep the PE busy while
the big activation images stream in (activation DMAs are chunked per
4 k-tiles and issued from the otherwise-idle Activation queue so the
sync queue's ~600ns/DMA issue serialization doesn't gate startup), and
the later hadamards multiply PSUM x SBUF directly with no bounce copy.
The U1 stream is emitted between H1's DVE muls and dependent work to
fill that stall. C3+U3 share one open PSUM accumulation group. PSUM
budget: 2 tags x 4 slots x 1 bank (two 256-wide h-chunks per bank tile)
= all 8 banks.

The two programs are dispatched concurrently on disjoint device subsets
through a copy of concourse's PJRT runner that takes an explicit device
list (the stock one hardcodes jax.devices()[:n]).
"""

import numpy as np

import jax
from jax.experimental.shard_map import shard_map
from jax.sharding import Mesh, PartitionSpec

import concourse.mybir as mybir
import concourse.tile as tile
from concourse import bacc, bass2jax

B, X, V, MM, VH, H1, H2, G = 256, 12000, 4096, 1024, 1024, 1024, 1024, 4
XP = 12032  # X padded to a multiple of 128 (94 k-tiles)
N_CORES = 8
NJ = H1 // 128  # output h-chunks per stream

DT_NAME = "bfloat16"  # matmul dtype: "float32r" or "bfloat16"

_cache = {}


def _mm_dt():
    return getattr(mybir.dt, DT_NAME)


def _np_dt():
    return mybir.dt.np(_mm_dt())


def build_program(role):
    """role "vis": visual+recurrent paths; "inp": input path. Full batch."""
    dt = _mm_dt()
    f32 = mybir.dt.float32

    nc = bacc.Bacc("TRN2", target_bir_lowering=False, debug=False)

    if role == "vis":
        act_specs = {"mT": MM, "hT": H2, "v1T": V, "v2T": V}
        w_specs = {"V1": V, "V2": V, "C1": VH, "C2": MM, "C3": H1,
                   "U1": H2, "U2": MM, "U3": H1}
    else:
        act_specs = {"mT": MM, "xT": XP}
        w_specs = {"W1": XP, "W2": MM, "W3": H1}

    acts_d = {
        name: nc.dram_tensor(name, [128, k // 128 * B], dt, kind="ExternalInput")
        for name, k in act_specs.items()
    }
    # weights arrive host-interleaved as k-tile pairs: [K/2, 2*H1], so a
    # [128, 2*H1] tile DMA moves 4KB contiguous per partition
    wt = {
        name: nc.dram_tensor(name, [k // 2, 2 * H1], dt, kind="ExternalInput")
        for name, k in w_specs.items()
    }
    out = nc.dram_tensor("out", [H1, B], f32, kind="ExternalOutput")

    with tile.TileContext(nc) as tc:
        with (
            tc.tile_pool(name="acts", bufs=1) as acts,
            tc.tile_pool(name="wstream", bufs=12) as wstream,
            tc.tile_pool(name="inter", bufs=1) as inter,
            tc.tile_pool(name="ps", bufs=4, space="PSUM") as ps,
        ):
            # PE warmup: ~32 dependency-free matmuls on zeroed SBUF start
            # the DVFS ramp during the DMA head instead of after it
            warm = acts.tile([128, 2 * B], dt, tag="warm")
            nc.vector.memset(warm[:], 0.0)
            wps = ps.tile([128, 2 * B], f32, tag="A", name="warm_ps")
            for i in range(16):
                nc.tensor.matmul(
                    wps[:, :B], warm[:, :128], warm[:, :B],
                    start=(i == 0), stop=(i == 15),
                )

            act_sb = {}

            def load_act(name, chunk_kt=4):
                """Chunked resident activation load, [128, ktile, batch]
                image; issued on the Activation queue."""
                dram = acts_d[name]
                ktiles = act_specs[name] // 128
                t = acts.tile([128, ktiles * B], dt, tag=name, name=name)
                for c0 in range(0, ktiles, chunk_kt):
                    c1 = min(c0 + chunk_kt, ktiles)
                    nc.scalar.dma_start(
                        t[:, c0 * B:c1 * B], dram.ap()[:, c0 * B:c1 * B]
                    )
                act_sb[name] = t.rearrange("p (t x) -> p t x", x=B)

            def act_view(name):
                return lambda k: act_sb[name][:, k, :]

            def q_view(q):
                return lambda k: q[:, k * B:(k + 1) * B]

            # psum: [128, 512] f32 bank tiles, two 256-wide h-chunks each
            def pslice(psums, j):
                return psums[j // 2][:, (j % 2) * B:(j % 2 + 1) * B]

            def stream_mm(rhs, wname, ptag, psums=None, start_group=True,
                          stop_group=True, j_outer=False):
                """pslice(psums, j) [128, 256] (+)= W_ktile[:, j*128:...]^T
                @ rhs(k), streaming W k-tiles: weight columns stationary,
                activation image moving. Weights arrive as k-tile PAIRS
                (host-interleaved so each DMA descriptor is a contiguous
                4KB). j_outer completes output chunks progressively (for
                the final stream, so evac/store overlap the matmul tail)
                with all pair-tiles staged first."""
                ktiles = w_specs[wname] // 128
                w_dram = wt[wname].ap().rearrange("(t p) n -> t p n", p=128)
                if psums is None:
                    psums = [
                        ps.tile([128, 2 * B], f32, tag=ptag,
                                name=f"ps_{wname}{i}")
                        for i in range(NJ // 2)
                    ]

                def mm(k, j, w):
                    nc.tensor.matmul(
                        pslice(psums, j),
                        w[:, (k % 2) * H1 + j * 128:(k % 2) * H1 + (j + 1) * 128],
                        rhs(k),
                        # start zeroes the whole 2KB PSUM bank, so only the
                        # first write of each bank pair may set it
                        start=start_group and (k == 0) and (j % 2 == 0),
                        stop=stop_group and (k == ktiles - 1),
                    )

                if j_outer:
                    ws = []
                    for k2 in range(ktiles // 2):
                        w = wstream.tile([128, 2 * H1], dt, tag="w",
                                         name=f"w_{wname}{k2}")
                        nc.sync.dma_start(w[:], w_dram[k2])
                        ws.append(w)
                    for j in range(NJ):
                        for k in range(ktiles):
                            mm(k, j, ws[k // 2])
                else:
                    for k2 in range(ktiles // 2):
                        w = wstream.tile([128, 2 * H1], dt, tag="w",
                                         name=f"w_{wname}{k2}")
                        nc.sync.dma_start(w[:], w_dram[k2])
                        for k in (2 * k2, 2 * k2 + 1):
                            for j in range(NJ):
                                mm(k, j, w)
                return psums

            def evac_sbuf(psums, name):
                """Copy psum accumulators to a resident SBUF f32 image,
                alternating Vector/Activation engines to halve latency."""
                s = inter.tile([128, NJ * B], f32, tag=name, name=name)
                for j in range(NJ):
                    dst = s[:, j * B:(j + 1) * B]
                    if j % 2 == 0:
                        nc.vector.tensor_copy(dst, pslice(psums, j))
                    else:
                        nc.scalar.copy(dst, pslice(psums, j))
                return s

            def had_mul(pa, partner_sb=None, bounce_from=None, qname="q"):
                """q [128, NJ*256] bf16 = pa * partner, chunk-granular.
                partner: resident SBUF f32 image, or psum bounced via SBUF."""
                q = inter.tile([128, NJ * B], dt, tag="q", name=qname, bufs=2)
                bnc = None
                if partner_sb is None:
                    bnc = inter.tile([128, NJ * B], f32, tag="bounce",
                                     name=f"bounce_{qname}", bufs=2)
                for j in range(NJ):
                    sl = slice(j * B, (j + 1) * B)
                    if partner_sb is None:
                        # bounce on the Activation engine so the Vector muls
                        # pipeline right behind it
                        nc.scalar.copy(bnc[:, sl], pslice(bounce_from, j))
                        src = bnc[:, sl]
                    else:
                        src = partner_sb[:, sl]
                    nc.vector.tensor_mul(q[:, sl], pslice(pa, j), src)
                return q

            def finish(l3):
                acc = inter.tile([128, NJ * B], f32, tag="acc", name="acc")
                half = NJ // 2 * B
                out_3d = out.ap().rearrange("(j p) b -> p j b", p=128)
                acc_3d = acc.rearrange("p (j b) -> p j b", b=B)
                for j in range(NJ):
                    sl = slice(j * B, (j + 1) * B)
                    if j % 2 == 0:
                        nc.vector.tensor_copy(acc[:, sl], pslice(l3, j))
                    else:
                        nc.scalar.copy(acc[:, sl], pslice(l3, j))
                    if j == NJ // 2 - 1:
                        # one fused store per half on separate queues: two
                        # ~600ns issues instead of eight serialized ones
                        nc.sync.dma_start(out_3d[:, :NJ // 2], acc_3d[:, :NJ // 2])
                nc.scalar.dma_start(out_3d[:, NJ // 2:], acc_3d[:, NJ // 2:])

            if role == "vis":
                load_act("mT")
                pc2 = stream_mm(act_view("mT"), "C2", "A")
                c2sb = evac_sbuf(pc2, "c2sb")
                load_act("hT")
                pu2 = stream_mm(act_view("mT"), "U2", "B")
                u2sb = evac_sbuf(pu2, "u2sb")
                load_act("v1T", chunk_kt=8)
                load_act("v2T", chunk_kt=8)
                pa = stream_mm(act_view("v1T"), "V1", "A")
                pb = stream_mm(act_view("v2T"), "V2", "B")
                q1 = had_mul(pa, bounce_from=pb, qname="q1")  # frees A and B
                # U1 in B: only waits H1's bounce copies, not its muls
                pu = stream_mm(act_view("hT"), "U1", "B")     # fills H1 stall
                pa2 = stream_mm(q_view(q1), "C1", "A")
                q2 = had_mul(pa2, partner_sb=c2sb, qname="q2")  # frees A
                l3 = stream_mm(q_view(q2), "C3", "A", start_group=True,
                               stop_group=False)
                q3 = had_mul(pu, partner_sb=u2sb, qname="q3")  # frees B; || C3
                stream_mm(q_view(q3), "U3", None, psums=l3,
                          start_group=False, stop_group=True, j_outer=True)
                finish(l3)
            else:
                load_act("mT")
                pw2 = stream_mm(act_view("mT"), "W2", "A")
                w2sb = evac_sbuf(pw2, "w2sb")
                load_act("xT", chunk_kt=8)
                pa = stream_mm(act_view("xT"), "W1", "B")
                q = had_mul(pa, partner_sb=w2sb, qname="q1")   # frees B
                # k-inner: W3 row k consumes q chunk k as the muls emit them
                l3 = stream_mm(q_view(q), "W3", "A")
                finish(l3)

    nc.compile()
    return nc


def _make_runner(nc, devices):
    """Adapted from concourse.bass2jax.run_bass_via_pjrt: same lowering,
    but runs on an explicit device subset and returns unmaterialized jax
    arrays so two programs can be dispatched concurrently."""
    bass2jax.install_neuronx_cc_hook()

    assert nc.dbg_addr is None
    partition_name = (
        nc.partition_id_tensor.name if nc.partition_id_tensor else None
    )

    in_names, out_names, out_avals, zero_outs = [], [], [], []
    for alloc in nc.m.functions[0].allocations:
        if not isinstance(alloc, mybir.MemoryLocationSet):
            continue
        name = alloc.memorylocations[0].name
        if alloc.kind == "ExternalInput":
            if name != partition_name:
                in_names.append(name)
        elif alloc.kind == "ExternalOutput":
            shape = tuple(alloc.tensor_shape)
            dtype = mybir.dt.np(alloc.dtype)
            out_names.append(name)
            out_avals.append(jax.core.ShapedArray(shape, dtype))
            zero_outs.append(np.zeros(shape, dtype))
    n_params = len(in_names)
    n_outs = len(out_avals)
    in_names.extend(out_names)
    if partition_name is not None:
        in_names.append(partition_name)
    donate = tuple(range(n_params, n_params + n_outs))

    def _body(*args):
        operands = list(args)
        if partition_name is not None:
            operands.append(bass2jax.partition_id_tensor())
        outs = bass2jax._bass_exec_p.bind(
            *operands,
            out_avals=tuple(out_avals),
            in_names=tuple(in_names),
            out_names=tuple(out_names),
            lowering_input_output_aliases=(),
            sim_require_finite=True,
            sim_require_nnan=True,
            nc=nc,
        )
        return tuple(outs)

    n_cores = len(devices)
    mesh = Mesh(np.asarray(devices), ("core",))
    in_specs = (PartitionSpec("core"),) * (n_params + n_outs)
    out_specs = (PartitionSpec("core"),) * n_outs
    sharded = jax.jit(
        shard_map(
            _body, mesh=mesh, in_specs=in_specs, out_specs=out_specs,
            check_rep=False,
        ),
        donate_argnums=donate,
        keep_unused=True,
    )

    def run(in_maps):
        assert len(in_maps) == n_cores
        concat_in = [
            np.concatenate(
                [np.asarray(in_maps[c][name]) for c in range(n_cores)], axis=0
            )
            for name in in_names[:n_params]
        ]
        concat_zeros = [
            np.zeros((n_cores * z.shape[0], *z.shape[1:]), z.dtype)
            for z in zero_outs
        ]
        out_arrs = sharded(*concat_in, *concat_zeros)
        return out_names, out_avals, out_arrs

    return run


def _wpair(w):
    """[K, H] -> [K/2, 2H]: interleave k-tile pairs so each partition's DMA
    line is 4KB contiguous (rows p and p+128 of a tile pair adjacent)."""
    wk = np.asarray(w, np.float32)
    K_, H = wk.shape
    kt = K_ // 128
    r = wk.reshape(kt // 2, 2, 128, H).transpose(0, 2, 1, 3)
    return np.ascontiguousarray(r.reshape(K_ // 2, 2 * H), dtype=_np_dt())


def _tile_actT(a, kdim):
    """[256 batch, K<=kdim] -> SBUF image [128, (kdim/128) * 256]:
    (p, t*256+b) = a[b, t*128+p], contiguous per partition."""
    ktiles = kdim // 128
    a = np.asarray(a, np.float32)
    if a.shape[1] < kdim:
        a = np.pad(a, ((0, 0), (0, kdim - a.shape[1])))
    # [256b, ktiles, 128p] -> [128p, ktiles, 256b]
    r = a.reshape(B, ktiles, 128).transpose(2, 1, 0)
    return np.ascontiguousarray(r.reshape(128, ktiles * B), dtype=_np_dt())


def kernel(prev_h, prev_c, x, m, v1, v2, V1, V2, C1, C2, C3, W1, W2, W3, U1, U2, U3, b):
    npdt = _np_dt()
    if "runners" not in _cache:
        devs = jax.devices()
        nc_vis = build_program("vis")
        nc_inp = build_program("inp")
        _cache["runners"] = (
            _make_runner(nc_vis, devs[0:4]),
            _make_runner(nc_inp, devs[4:8]),
        )
        _cache["ncs"] = (nc_vis, nc_inp)
    run_vis, run_inp = _cache["runners"]

    v1T_img = _tile_actT(v1, V)
    v2T_img = _tile_actT(v2, V)
    mT_img = _tile_actT(m, MM)
    hT_img = _tile_actT(prev_h, H2)
    xT_img = _tile_actT(x, XP)

    vis_maps, inp_maps = [], []
    for g in range(G):
        vis_maps.append({
            "v1T": v1T_img, "v2T": v2T_img, "mT": mT_img, "hT": hT_img,
            "V1": _wpair(V1[g]), "V2": _wpair(V2[g]), "C1": _wpair(C1[g]),
            "C2": _wpair(C2[g]), "C3": _wpair(C3[g]), "U1": _wpair(U1[g]),
            "U2": _wpair(U2[g]), "U3": _wpair(U3[g]),
        })
        w1_pad = np.zeros((XP, H1), np.float32)
        w1_pad[:X] = np.asarray(W1[g], np.float32)
        inp_maps.append({
            "xT": xT_img, "mT": mT_img,
            "W1": _wpair(w1_pad),
            "W2": _wpair(W2[g]), "W3": _wpair(W3[g]),
        })

    _cache["last_in_maps"] = (vis_maps, inp_maps)

    # dispatch both programs; they run concurrently on disjoint cores
    vnames, vavals, vouts = run_vis(vis_maps)
    inames, iavals, iouts = run_inp(inp_maps)

    # outputs are logits^T [G, H2, B]
    vis_out = np.asarray(vouts[0]).reshape(G, H1, B)
    inp_out = np.asarray(iouts[0]).reshape(G, H1, B)

    logits = (vis_out + inp_out).transpose(0, 2, 1) + \
        np.asarray(b, np.float32)[:, None, :]

    def sigmoid(z):
        return 1.0 / (1.0 + np.exp(-z))

    i = sigmoid(logits[0])
    f = sigmoid(logits[1])
    o = sigmoid(logits[2])
    cg = np.tanh(logits[3])
    prev_c = np.asarray(prev_c, np.float32)
    new_c = f * prev_c + i * cg
    new_h = o * np.tanh(prev_c)
    return new_h.astype(np.float32), new_c.astype(np.float32)
